# revision 6
# baseline (speedup 1.0000x reference)
"""Trainium2 Bass kernel for nn_MultiHeadAttention (decode-style, q_len=1).

Data-parallel over batch: 64 batches -> 8 cores x 8 batches.

Key algebraic restructuring (exact, exploits q_len == 1):
  scores[b,h,s] = (q Wq + bq)_h . (k Wk + bk)_h
                = k[b,s,:] . R_b[:,h] + const(b,h)        # const drops in softmax
     where R_b[d,h] = sum_{d'} Wk[d, h*64+d'] qh[b, h*64+d']
  out_concat[b,hd] = (sum_s p[b,h,s] v[b,s,:]) @ Wv[:,hd] + bv[hd]
so the big K/V projections (2 x 275 GFLOP) are never computed; instead
k and v are contracted directly (2 x 4.3 GFLOP) and the kernel becomes
HBM-bound on streaming k,v (128 MiB/core).

Matmuls on the big streams use float32r (TF32-like, ~1e-4 rel err).
k is transposed on-chip via PE transposes; rounding to f32r rides the
mandatory PSUM->SBUF copies.
"""

import numpy as np
from contextlib import ExitStack

import concourse.bass as bass
import concourse.tile as tile
from concourse import bacc, mybir
from concourse.bass_utils import run_bass_kernel_spmd

try:
    import axon_profile_shim
    axon_profile_shim.install()
except Exception:
    pass

N_CORES = 8
D = 1024
H = 16
DK = 64
F32 = mybir.dt.float32
F32R = mybir.dt.float32r
AX = mybir.AxisListType
ALU = mybir.AluOpType
ACTF = mybir.ActivationFunctionType


def _make_identity(nc, ap):
    nc.gpsimd.memset(ap, 0.0)
    nc.gpsimd.affine_select(
        out=ap, in_=ap, compare_op=ALU.not_equal, fill=1.0,
        base=0, pattern=[[-1, ap.shape[0]]], channel_multiplier=1,
    )


def build(BL=8, S=2048, n_cores=N_CORES):
    """Build + compile the per-core program. BL = local batches, S = seq len."""
    SC = S // 128          # 128-row s-subchunks
    SG = S // 512          # 512-row s-groups
    nc = bacc.Bacc("TRN2", target_bir_lowering=False, debug=False,
                   num_devices=n_cores)

    q_ext = nc.dram_tensor("q", [BL, D], F32, kind="ExternalInput").ap()
    k_ext = nc.dram_tensor("k", [BL * S, D], F32, kind="ExternalInput").ap()
    v_ext = nc.dram_tensor("v", [BL * S, D], F32, kind="ExternalInput").ap()
    Wq_ext = nc.dram_tensor("Wq", [D, D], F32, kind="ExternalInput").ap()
    Wk_ext = nc.dram_tensor("Wk", [D, D], F32, kind="ExternalInput").ap()
    Wv_ext = nc.dram_tensor("Wv", [D, D], F32, kind="ExternalInput").ap()
    Wo_ext = nc.dram_tensor("Wo", [D, D], F32, kind="ExternalInput").ap()
    bq_ext = nc.dram_tensor("bq", [D], F32, kind="ExternalInput").ap()
    bv_ext = nc.dram_tensor("bv", [D], F32, kind="ExternalInput").ap()
    bo_ext = nc.dram_tensor("bo", [D], F32, kind="ExternalInput").ap()
    y_ext = nc.dram_tensor("y", [BL, D], F32, kind="ExternalOutput").ap()

    with tile.TileContext(nc) as tc, ExitStack() as ctx:
        cpool = ctx.enter_context(tc.tile_pool(name="const", bufs=1))
        ident = cpool.tile([128, 128], F32)
        _make_identity(nc, ident[:])
        bq_sb = cpool.tile([128, 8], F32)
        nc.sync.dma_start(bq_sb[:], bq_ext.rearrange("(m p) -> p m", p=128))
        bv8 = cpool.tile([BL, D], F32)
        nc.sync.dma_start(bv8[:], bv_ext.unsqueeze(0).broadcast_to([BL, D]))
        bo8 = cpool.tile([BL, D], F32)
        nc.sync.dma_start(bo8[:], bo_ext.unsqueeze(0).broadcast_to([BL, D]))

        # persistent across whole kernel
        zeros32 = cpool.tile([128, 128], F32)
        nc.vector.memset(zeros32[:], 0.0)
        R_sb = [cpool.tile([128, 8, H], F32R, tag=f"R{b}", name=f"R{b}") for b in range(BL)]
        UT_all = cpool.tile([128, 8, H, BL], F32)

        # Wv (fp32 raw) + Wo (rounded to f32r) for the tail projections
        Wv_sb = [cpool.tile([128, D], F32, tag=f"wv{j}", name=f"wv{j}") for j in range(8)]
        Wo_r = [cpool.tile([128, D], F32R, tag=f"wor{j}", name=f"wor{j}") for j in range(8)]

        # ---------------- setup: qh^T, Wk^T, R ----------------
        with tc.tile_pool(name="wsetup", bufs=1) as wpool, \
             tc.tile_pool(name="spsum", bufs=1, space="PSUM") as spsum:
            Q = wpool.tile([BL, D], F32)
            nc.sync.dma_start(Q[:], q_ext[:])

            qtp = spsum.tile([128, 8 * BL], F32, tag="qtp")
            for i in range(8):
                nc.tensor.transpose(qtp[:, i * BL:(i + 1) * BL],
                                    Q[:, i * 128:(i + 1) * 128], ident[:BL, :BL])
            QT_sb = wpool.tile([128, 8 * BL], F32)
            nc.vector.tensor_copy(QT_sb[:], qtp[:])

            Wq_sb = [wpool.tile([128, D], F32, tag=f"wq{i}", name=f"wq{i}") for i in range(8)]
            for i in range(8):
                nc.sync.dma_start(Wq_sb[i][:], Wq_ext[i * 128:(i + 1) * 128, :])
            qhT_sb = wpool.tile([128, 8 * BL], F32)  # [p, m*BL + b]
            for m in range(8):
                qp = spsum.tile([128, BL], F32, tag="qhp")
                for i in range(8):
                    nc.tensor.matmul(qp[:], Wq_sb[i][:, m * 128:(m + 1) * 128],
                                     QT_sb[:, i * BL:(i + 1) * BL],
                                     start=(i == 0), stop=(i == 7))
                nc.vector.tensor_scalar_add(qhT_sb[:, m * BL:(m + 1) * BL],
                                            qp[:], bq_sb[:, m:m + 1])

            Wk_sb = [wpool.tile([128, D], F32, tag=f"wk{a}", name=f"wk{a}") for a in range(8)]
            for a in range(8):
                nc.sync.dma_start(Wk_sb[a][:], Wk_ext[a * 128:(a + 1) * 128, :])
            WkT = [wpool.tile([128, D], F32R, tag=f"wkt{c}", name=f"wkt{c}") for c in range(8)]
            for c in range(8):
                wp = spsum.tile([128, D], F32, tag="wtp")
                for a in range(8):
                    nc.tensor.transpose(wp[:, a * 128:(a + 1) * 128],
                                        Wk_sb[a][:, c * 128:(c + 1) * 128],
                                        ident[:])
                nc.vector.tensor_copy(WkT[c][:], wp[:])

            for b in range(BL):
                qb = wpool.tile([128, 8, H], F32R, tag="qblk")
                nc.vector.tensor_copy(qb[:].rearrange("p a b -> p (a b)"), zeros32[:, :8 * H])
                for c in range(8):
                    nc.vector.tensor_copy(qb[0:64, c, 2 * c:2 * c + 1],
                                          qhT_sb[0:64, c * BL + b:c * BL + b + 1])
                    nc.vector.tensor_copy(qb[64:128, c, 2 * c + 1:2 * c + 2],
                                          qhT_sb[64:128, c * BL + b:c * BL + b + 1])
                rtp = spsum.tile([H, D], F32, tag="rtp")
                for n in range(2):
                    for c in range(8):
                        nc.tensor.matmul(rtp[:, n * 512:(n + 1) * 512],
                                         qb[:, c, :],
                                         WkT[c][:, n * 512:(n + 1) * 512],
                                         start=(c == 0), stop=(c == 7))
                RT_b = wpool.tile([H, D], F32, tag="rt")
                nc.vector.tensor_copy(RT_b[:], rtp[:])
                rp = spsum.tile([128, 8 * H], F32, tag="rp")
                for d in range(8):
                    nc.tensor.transpose(rp[:, d * H:(d + 1) * H],
                                        RT_b[:, d * 128:(d + 1) * 128],
                                        ident[:H, :H])
                nc.vector.tensor_copy(
                    R_sb[b][:], rp[:].rearrange("p (d h) -> p d h", d=8))

        # load tail weights (scheduler places these when SBUF frees up)
        for j in range(8):
            nc.sync.dma_start(Wv_sb[j][:], Wv_ext[j * 128:(j + 1) * 128, :])
        with tc.tile_pool(name="woraw", bufs=2) as wopool:
            for j in range(8):
                wo_raw = wopool.tile([128, D], F32, tag="woraw")
                nc.sync.dma_start(wo_raw[:], Wo_ext[j * 128:(j + 1) * 128, :])
                nc.vector.tensor_copy(Wo_r[j][:], wo_raw[:])

        # ---------------- stream phase ----------------
        kpool = ctx.enter_context(tc.tile_pool(name="kpool", bufs=4))
        ktpool = ctx.enter_context(tc.tile_pool(name="ktpool", bufs=2))
        vpool = ctx.enter_context(tc.tile_pool(name="vpool", bufs=6))
        vrpool = ctx.enter_context(tc.tile_pool(name="vrpool", bufs=2))
        epool = ctx.enter_context(tc.tile_pool(name="epool", bufs=2))
        etpool = ctx.enter_context(tc.tile_pool(name="etpool", bufs=2))
        upool = ctx.enter_context(tc.tile_pool(name="upool", bufs=2))
        stream_psum = ExitStack()
        tpp = stream_psum.enter_context(tc.tile_pool(name="tpp", bufs=3, space="PSUM"))
        scp = stream_psum.enter_context(tc.tile_pool(name="scp", bufs=2, space="PSUM"))
        upp = stream_psum.enter_context(tc.tile_pool(name="upp", bufs=1, space="PSUM"))

        for b in range(BL):
            E_b = epool.tile([H, S], F32, tag="E")
            den4 = epool.tile([H, SG], F32, tag="den4")
            for g in range(SG):
                kt4 = ktpool.tile([128, 8, 512], F32R, tag="kt4")
                for c in range(4):
                    kc = kpool.tile([128, D], F32, tag="kc")
                    r0 = b * S + g * 512 + c * 128
                    nc.sync.dma_start(kc[:], k_ext[r0:r0 + 128, :])
                    for half in range(2):
                        tp = tpp.tile([128, 512], F32, tag="tp")
                        for j4 in range(4):
                            j = half * 4 + j4
                            nc.tensor.transpose(tp[:, j4 * 128:(j4 + 1) * 128],
                                                kc[:, j * 128:(j + 1) * 128],
                                                ident[:])
                        nc.vector.tensor_copy(
                            kt4[:, half * 4:(half + 1) * 4, c * 128:(c + 1) * 128],
                            tp[:].rearrange("p (j s) -> p j s", j=4))
                sc = scp.tile([H, 512], F32, tag="sc")
                for j in range(8):
                    nc.tensor.matmul(sc[:], R_sb[b][:, j, :], kt4[:, j, :],
                                     start=(j == 0), stop=(j == 7))
                nc.scalar.activation(E_b[:, g * 512:(g + 1) * 512], sc[:],
                                     ACTF.Exp, scale=0.125,
                                     accum_out=den4[:, g:g + 1])

            den = epool.tile([H, 1], F32, tag="den")
            nc.vector.tensor_reduce(den[:], den4[:], axis=AX.X, op=ALU.add)
            rden = epool.tile([H, 1], F32, tag="rden")
            nc.vector.reciprocal(rden[:], den[:])

            ET_b = etpool.tile([128, SC, H], F32R, tag="ET")
            gsz = min(8, SC)
            for tg in range(SC // gsz):
                sp = tpp.tile([128, gsz * H], F32, tag="tp", name="sp")
                for i in range(gsz):
                    t = tg * gsz + i
                    nc.tensor.transpose(sp[:, i * H:(i + 1) * H],
                                        E_b[:, t * 128:(t + 1) * 128],
                                        ident[:H, :H])
                nc.vector.tensor_copy(
                    ET_b[:, tg * gsz:(tg + 1) * gsz, :],
                    sp[:, :gsz * H].rearrange("p (t h) -> p t h", t=gsz))

            up = upp.tile([H, D], F32, tag="up")
            for t in range(SC):
                vc = vpool.tile([128, D], F32, tag="vc")
                nc.sync.dma_start(vc[:], v_ext[b * S + t * 128:b * S + t * 128 + 128, :])
                vr = vrpool.tile([128, D], F32R, tag="vr")
                nc.vector.tensor_copy(vr[:], vc[:])
                for n in range(2):
                    nc.tensor.matmul(up[:, n * 512:(n + 1) * 512],
                                     ET_b[:, t, :], vr[:, n * 512:(n + 1) * 512],
                                     start=(t == 0), stop=(t == SC - 1))
            U_sb = upool.tile([H, D], F32, tag="U")
            nc.vector.tensor_scalar_mul(U_sb[:], up[:], rden[:])

            sp = tpp.tile([128, 8 * H], F32, tag="tp")
            for jc in range(8):
                nc.tensor.transpose(sp[:, jc * H:(jc + 1) * H],
                                    U_sb[:, jc * 128:(jc + 1) * 128],
                                    ident[:H, :H])
            nc.vector.tensor_copy(
                UT_all[:, :, :, b],
                sp[:].rearrange("p (j h) -> p j h", j=8))

        # ---------------- tail: out-projection ----------------
        stream_psum.close()
        with tc.tile_pool(name="fin", bufs=1) as fpool, \
             tc.tile_pool(name="fpsum", bufs=1, space="PSUM") as fpsum:
            oc = fpsum.tile([BL, D], F32, tag="oc")
            for h in range(H):
                for jc in range(8):
                    nc.tensor.matmul(oc[:, h * 64:(h + 1) * 64],
                                     UT_all[:, jc, h, :],
                                     Wv_sb[jc][:, h * 64:(h + 1) * 64],
                                     start=(jc == 0), stop=(jc == 7))
            OC_sb = fpool.tile([BL, D], F32)
            nc.vector.tensor_add(OC_sb[:], oc[:], bv8[:])

            op = fpsum.tile([128, 8 * BL], F32, tag="op")
            for jc in range(8):
                nc.tensor.transpose(op[:, jc * BL:(jc + 1) * BL],
                                    OC_sb[:, jc * 128:(jc + 1) * 128],
                                    ident[:BL, :BL])
            OCT = fpool.tile([128, 8, BL], F32R)
            nc.vector.tensor_copy(OCT[:], op[:].rearrange("p (j b) -> p j b", j=8))

            yp = fpsum.tile([BL, D], F32, tag="yp")
            for n in range(2):
                for jc in range(8):
                    nc.tensor.matmul(yp[:, n * 512:(n + 1) * 512],
                                     OCT[:, jc, :],
                                     Wo_r[jc][:, n * 512:(n + 1) * 512],
                                     start=(jc == 0), stop=(jc == 7))
            ytmp = fpool.tile([BL, D], F32)
            nc.vector.tensor_add(ytmp[:], yp[:], bo8[:])
            y_sb = fpool.tile([BL, D], F32)
            nc.vector.tensor_scalar_max(y_sb[:], ytmp[:], 0.0)
            nc.sync.dma_start(y_ext[:], y_sb[:])

    nc.compile()
    return nc


_built = {}


def _get_nc(BL, S):
    key = (BL, S)
    if key not in _built:
        _built[key] = build(BL, S)
    return _built[key]


def kernel(q, k, v, Wq, bq, Wk, bk, Wv, bv, Wo, bo, _trace=False):
    q = np.asarray(q, dtype=np.float32)
    k = np.asarray(k, dtype=np.float32)
    v = np.asarray(v, dtype=np.float32)
    B, S = k.shape[0], k.shape[1]
    BL = B // N_CORES
    nc = _get_nc(BL, S)

    shared = {
        "Wq": np.ascontiguousarray(Wq, dtype=np.float32),
        "Wk": np.ascontiguousarray(Wk, dtype=np.float32),
        "Wv": np.ascontiguousarray(Wv, dtype=np.float32),
        "Wo": np.ascontiguousarray(Wo, dtype=np.float32),
        "bq": np.ascontiguousarray(bq, dtype=np.float32),
        "bv": np.ascontiguousarray(bv, dtype=np.float32),
        "bo": np.ascontiguousarray(bo, dtype=np.float32),
    }
    in_maps = []
    for c in range(N_CORES):
        sl = slice(c * BL, (c + 1) * BL)
        in_maps.append({
            "q": np.ascontiguousarray(q[sl].reshape(BL, D)),
            "k": np.ascontiguousarray(k[sl].reshape(BL * S, D)),
            "v": np.ascontiguousarray(v[sl].reshape(BL * S, D)),
            **shared,
        })
    res = run_bass_kernel_spmd(nc, in_maps, list(range(N_CORES)), trace=_trace)
    out = np.concatenate([res.results[c]["y"] for c in range(N_CORES)], axis=0)
    if _trace:
        kernel._last_exec_time_ns = res.exec_time_ns
        kernel._last_profile = res.profile_json
    return out
